# revision 10
# baseline (speedup 1.0000x reference)
"""Trainium2 Bass kernel v2 for nn_BaseLineModel (segment_reduce).

Changes vs v1 baseline:
- Embedding gather uses NON-transpose dma_gather of exact 256B fp32 rows
  (token -> one contiguous 256B descriptor) instead of transpose-mode
  (which wrote 2B per partition per token -> read-modify-write death,
  ~670ms/pass). Token-major gather output is transposed to feature-major
  on-device with paired PE transposes; the k0|k1 stacked bf16 conv
  operand is built during the PSUM->SBUF copies (all on ACT; DVE is the
  busier engine with the maxpool). Gathers ride 4 SWDGE queues
  (multi-packet, 8192 idx/call) - queue count scales random-read BW.
- The per-day linear layer is applied BEFORE the segment reduction:
  per-note dot = [delta|feats].W computed on DVE, then segment-sum of
  [dot, 1] via one-hot matmuls -> [2, 1024] partials, AllReduce (8KB)
  instead of ReduceScatter of [1024, 258] (1MB).
- Finalize: mean = dot_sum/max(cnt,1), sigmoid(mean + b) on a [1, 1024]
  row; host takes core 0's full output.
"""

import numpy as np
import ml_dtypes

import concourse.bass as bass
import concourse.mybir as mybir
import concourse.tile as tile
from concourse.bass_utils import run_bass_kernel_spmd
from concourse import library_config
from concourse.masks import make_identity

# ---- problem dims (hardcoded per task contract) ----
N, L, E, H, K, V, S = 16384, 64, 64, 256, 3, 30000, 1024
NCORES = 8
NC_NOTES = N // NCORES            # 2048 notes per core
NTOK = NC_NOTES * L               # 131072 tokens per core
BLK_TOK = 8192                    # tokens per processing block
NBLK = NTOK // BLK_TOK            # 16
GCHUNK = 8192                     # tokens per dma_gather call
NGC = BLK_TOK // GCHUNK           # gather calls per block
# >64 descs/engine (GCHUNK>1008) cannot ride one packet; use multi-packet then
SINGLE_PACKET = (GCHUNK // 16 + 1) <= 64
NSUB = BLK_TOK // 128             # 64 sub-blocks of 128 tokens
NPAIR = NSUB // 2                 # 32 transpose pairs
NGRP = BLK_TOK // 512             # 16 conv groups per block
NCHUNK = NC_NOTES // 128          # 16 note-chunks
TMAX = L - K + 1                  # 62 valid conv positions

_SPLIT_MAXW = 1


def _split_waits(nc, maxw=_SPLIT_MAXW):
    """This walrus build rejects >1 sync wait per instruction; move extras
    onto preceding same-engine NOPs (sequencer order preserves semantics)."""
    for bb in nc.main_func.blocks:
        out = []
        for inst in bb.instructions:
            si = inst.sync_info
            waits = list(si.on_wait) if (si is not None and si.on_wait) else []
            if len(waits) > maxw:
                rest = waits[:-maxw]
                si.on_wait = waits[-maxw:]
                for i in range(0, len(rest), maxw):
                    out.append(mybir.InstNoOp(
                        name=f"{inst.name}-wsplit{i}",
                        sync_info=mybir.SyncInfo(on_wait=rest[i:i + maxw], on_update=[]),
                        bass_nofuse=True,
                        engine=inst.engine,
                    ))
            out.append(inst)
        bb.instructions = out


def _build_nc(reps=1, use_cc=True, mode="dev", postprocess=True):
    f32 = mybir.dt.float32
    bf16 = mybir.dt.bfloat16
    i16 = mybir.dt.int16
    Relu = mybir.ActivationFunctionType.Relu
    Copy = mybir.ActivationFunctionType.Copy
    Sigmoid = mybir.ActivationFunctionType.Sigmoid

    nc = bass.Bass(num_swdge_queues=4 if mode == "dev" else 1)
    if mode == "dev":
        d_embf = nc.declare_dram_parameter("embf", [V, E], f32, isOutput=False)
        d_idx = nc.declare_dram_parameter("idx", [128, NTOK // 16], i16, isOutput=False)
    else:
        d_x = nc.declare_dram_parameter("xs", [NBLK, 128, BLK_TOK], bf16, isOutput=False)
    d_stf = nc.declare_dram_parameter("stf", [128, NCHUNK], f32, isOutput=False)
    d_delta = nc.declare_dram_parameter("delta", [128, NCHUNK], f32, isOutput=False)
    d_w01 = nc.declare_dram_parameter("w01", [128, H], bf16, isOutput=False)
    d_w2 = nc.declare_dram_parameter("w2", [64, H], bf16, isOutput=False)
    d_cb = nc.declare_dram_parameter("convb2", [128, 2], f32, isOutput=False)
    d_iota = nc.declare_dram_parameter("iota", [128, S], f32, isOutput=False)
    d_wf = nc.declare_dram_parameter("wf", [128, H], f32, isOutput=False)
    d_w0 = nc.declare_dram_parameter("w0", [128, 1], f32, isOutput=False)
    d_bsc = nc.declare_dram_parameter("bsc", [1, 1], f32, isOutput=False)
    d_out = nc.declare_dram_parameter("out", [1, S], f32, isOutput=True)
    part = nc.dram_tensor("part", [2, S], f32)
    ar_out = nc.dram_tensor("ar_out", [2, S], f32)

    with tile.TileContext(nc) as tc:
        if mode == "dev":
            nc.gpsimd.load_library(library_config.mlp)
            nidx_reg_cm = nc.gpsimd.register("nidx")
            nidx_reg = nidx_reg_cm.__enter__()
            nc.gpsimd.reg_mov(nidx_reg, GCHUNK)
        with (
            tc.tile_pool(name="cst", bufs=1) as cp,
            tc.tile_pool(name="feat", bufs=1) as fp,
        ):
            w01_sb = cp.tile([128, H], bf16)
            w2_sb = cp.tile([64, H], bf16)
            cb_sb = cp.tile([128, 2], f32)
            iota_sb = cp.tile([128, S], f32)
            wf_sb = cp.tile([128, H], f32)
            w0_sb = cp.tile([128, 1], f32)
            bsc_sb = cp.tile([1, 1], f32)
            stf_sb = cp.tile([128, NCHUNK], f32)
            delta_sb = cp.tile([128, NCHUNK], f32)
            ones_sb = cp.tile([128, 1], f32)
            nc.vector.memset(ones_sb[:], 1.0)
            if mode == "dev":
                idx_sb = cp.tile([128, NTOK // 16], i16)
            ident = cp.tile([128, 128], f32)
            nc.sync.dma_start(out=w01_sb[:], in_=d_w01[:])
            nc.sync.dma_start(out=w2_sb[:], in_=d_w2[:])
            nc.sync.dma_start(out=cb_sb[:], in_=d_cb[:])
            nc.sync.dma_start(out=iota_sb[:], in_=d_iota[:])
            nc.sync.dma_start(out=wf_sb[:], in_=d_wf[:])
            nc.sync.dma_start(out=w0_sb[:], in_=d_w0[:])
            nc.sync.dma_start(out=bsc_sb[:], in_=d_bsc[:])
            nc.sync.dma_start(out=stf_sb[:], in_=d_stf[:])
            nc.sync.dma_start(out=delta_sb[:], in_=d_delta[:])
            if mode == "dev":
                nc.sync.dma_start(out=idx_sb[:], in_=d_idx[:])
            make_identity(nc, ident[:])

            for _rep in range(reps):
                feats = [fp.tile([128, NC_NOTES], f32, tag=f"feats{hh}",
                                 name=f"feats{hh}")
                         for hh in range(2)]

                # ---- P1: gather (token-major) + transpose-stack + conv + maxpool ----
                with (
                    tc.tile_pool(name="gath", bufs=4) as gp,
                    tc.tile_pool(name="xstk", bufs=2) as xp,
                    tc.tile_pool(name="tps", bufs=4, space="PSUM") as tp,
                    tc.tile_pool(name="ypsum", bufs=3, space="PSUM") as yp,
                ):
                    for b in range(NBLK):
                        if mode == "host":
                            xstack = xp.tile([128, BLK_TOK], bf16, tag="xs")
                            nc.sync.dma_start(out=xstack[:], in_=d_x[b])
                        else:
                            x_tok = gp.tile([128, NSUB * E], f32, tag="xtok")
                            for gc in range(NGC):
                                col0 = gc * (GCHUNK // 128) * E
                                icol0 = b * (BLK_TOK // 16) + gc * (GCHUNK // 16)
                                nc.gpsimd.dma_gather(
                                    out_ap=x_tok[:, col0:col0 + (GCHUNK // 128) * E]
                                        .rearrange("p (o n) -> p o n", n=E),
                                    in_ap=d_embf[:],
                                    idxs_ap=idx_sb[:, icol0:icol0 + GCHUNK // 16],
                                    num_idxs=GCHUNK,
                                    num_idxs_reg=nidx_reg,
                                    elem_size=E,
                                    transpose=False,
                                    single_packet=SINGLE_PACKET,
                                    queue_num=(b * NGC + gc) % 4,
                                )
                            xstack = xp.tile([128, BLK_TOK], bf16, tag="xs")
                            # last k1 column is never produced (needs next
                            # block); zero it so the discarded tail is finite
                            nc.vector.memset(xstack[64:128, BLK_TOK - 1:BLK_TOK], 0.0)
                            for k in range(NPAIR):
                                t_ps = tp.tile([128, 128], f32, tag="t")
                                nc.tensor.transpose(out=t_ps[:],
                                                    in_=x_tok[:, k * 128:(k + 1) * 128],
                                                    identity=ident[:])
                                c_e = 2 * k * 128       # even sub-block col base
                                c_o = c_e + 128         # odd sub-block col base
                                # k0 rows (ACT engine)
                                nc.scalar.activation(out=xstack[0:64, c_e:c_e + 128],
                                                     in_=t_ps[0:64, :], func=Copy)
                                nc.scalar.activation(out=xstack[0:64, c_o:c_o + 128],
                                                     in_=t_ps[64:128, :], func=Copy)
                                # k1 rows, shifted left by one token (ACT too:
                                # DVE is the busier engine with the maxpool)
                                if k == 0:
                                    nc.scalar.activation(out=xstack[64:128, 0:127],
                                                         in_=t_ps[0:64, 1:128],
                                                         func=Copy)
                                else:
                                    nc.scalar.activation(
                                        out=xstack[64:128, c_e - 1:c_e + 127],
                                        in_=t_ps[0:64, :], func=Copy)
                                nc.scalar.activation(
                                    out=xstack[64:128, c_o - 1:c_o + 127],
                                    in_=t_ps[64:128, :], func=Copy)
                        for g in range(NGRP):
                            c0 = g * 512
                            for hh in range(2):
                                y_ps = yp.tile([128, 512], f32, tag="y")
                                nc.tensor.matmul(out=y_ps[:],
                                                 lhsT=w01_sb[:, hh * 128:(hh + 1) * 128],
                                                 rhs=xstack[:, c0:c0 + 512],
                                                 start=True, stop=False)
                                nc.tensor.matmul(out=y_ps[:, 0:510],
                                                 lhsT=w2_sb[:, hh * 128:(hh + 1) * 128],
                                                 rhs=xstack[0:64, c0 + 2:c0 + 512],
                                                 start=False, stop=True)
                                nc.vector.reduce_max(
                                    out=feats[hh][:, b * 128 + g * 8:b * 128 + g * 8 + 8],
                                    in_=y_ps[:].rearrange("p (n l) -> p n l", l=L)[:, :, 0:TMAX],
                                    axis=mybir.AxisListType.X)

                # ---- P2: relu(feats + conv_b) ----
                for hh in range(2):
                    nc.scalar.activation(out=feats[hh][:], in_=feats[hh][:],
                                         func=Relu, bias=cb_sb[:, hh:hh + 1], scale=1.0)

                # ---- P3+P4: note-major dot, one-hot segment matmuls ----
                with (
                    tc.tile_pool(name="seg", bufs=1) as ssp,
                    tc.tile_pool(name="tps2", bufs=2, space="PSUM") as tp2,
                    tc.tile_pool(name="segps", bufs=1, space="PSUM") as pp,
                ):
                    seg_ps = [[pp.tile([1, 512], f32, tag=f"seg{v}{h}",
                                       name=f"seg{v}{h}")
                               for h in range(2)] for v in range(2)]
                    for i in range(NCHUNK):
                        mainf = ssp.tile([128, H], f32, tag="mainf")
                        for hh in range(2):
                            t2 = tp2.tile([128, 128], f32, tag="t2")
                            nc.tensor.transpose(out=t2[:],
                                                in_=feats[hh][:, i * 128:(i + 1) * 128],
                                                identity=ident[:])
                            nc.vector.tensor_copy(
                                out=mainf[:, hh * 128:(hh + 1) * 128], in_=t2[:])
                        prod = ssp.tile([128, H], f32, tag="prod")
                        nc.vector.tensor_tensor(out=prod[:], in0=mainf[:], in1=wf_sb[:],
                                                op=mybir.AluOpType.mult)
                        dotc = ssp.tile([128, 1], f32, tag="dotc")
                        nc.vector.reduce_sum(out=dotc[:], in_=prod[:],
                                             axis=mybir.AxisListType.X)
                        dterm = ssp.tile([128, 1], f32, tag="dterm")
                        nc.vector.tensor_tensor(out=dterm[:], in0=delta_sb[:, i:i + 1],
                                                in1=w0_sb[:], op=mybir.AluOpType.mult)
                        nc.vector.tensor_add(out=dotc[:], in0=dotc[:], in1=dterm[:])
                        oh = ssp.tile([128, S], f32, tag="oh")
                        nc.vector.tensor_tensor(out=oh[:],
                                                in0=stf_sb[:, i:i + 1].to_broadcast([128, S]),
                                                in1=iota_sb[:],
                                                op=mybir.AluOpType.is_equal)
                        for h in range(2):
                            nc.tensor.matmul(out=seg_ps[0][h][:],
                                             lhsT=dotc[:],
                                             rhs=oh[:, h * 512:(h + 1) * 512],
                                             start=(i == 0), stop=(i == NCHUNK - 1))
                            nc.tensor.matmul(out=seg_ps[1][h][:],
                                             lhsT=ones_sb[:],
                                             rhs=oh[:, h * 512:(h + 1) * 512],
                                             start=(i == 0), stop=(i == NCHUNK - 1))
                    sd = fp.tile([1, S], f32, tag="segd")
                    sc = fp.tile([1, S], f32, tag="segc")
                    for h in range(2):
                        nc.vector.tensor_copy(out=sd[:, h * 512:(h + 1) * 512],
                                              in_=seg_ps[0][h][:])
                        nc.vector.tensor_copy(out=sc[:, h * 512:(h + 1) * 512],
                                              in_=seg_ps[1][h][:])
                    nc.sync.dma_start(out=part[0:1, :], in_=sd[:])
                    nc.sync.dma_start(out=part[1:2, :], in_=sc[:])

                # ---- P5: cross-core reduce + finalize ----
                if use_cc:
                    with tc.tile_critical():
                        with nc.semaphore(f"cc_sem{_rep}") as cc_sem:
                            nc.gpsimd.collective_compute(
                                "AllReduce", mybir.AluOpType.add,
                                replica_groups=[list(range(NCORES))],
                                ins=[part[:]], outs=[ar_out[:]],
                            ).then_inc(cc_sem, 1)
                            nc.gpsimd.wait_ge(cc_sem, 1)
                with tc.tile_pool(name="fin", bufs=1) as fin:
                    if use_cc:
                        fd = fin.tile([1, S], f32)
                        fc = fin.tile([1, S], f32)
                        nc.sync.dma_start(out=fd[:], in_=ar_out[0:1, :])
                        nc.sync.dma_start(out=fc[:], in_=ar_out[1:2, :])
                    else:
                        fd, fc = sd, sc
                    cnt = fin.tile([1, S], f32)
                    nc.vector.tensor_scalar_max(out=cnt[:], in0=fc[:], scalar1=1.0)
                    rcp = fin.tile([1, S], f32)
                    nc.vector.reciprocal(out=rcp[:], in_=cnt[:])
                    v = fin.tile([1, S], f32)
                    nc.vector.tensor_tensor(out=v[:], in0=fd[:], in1=rcp[:],
                                            op=mybir.AluOpType.mult)
                    outsb = fin.tile([1, S], f32)
                    nc.scalar.activation(out=outsb[:], in_=v[:], func=Sigmoid,
                                         bias=bsc_sb[:], scale=1.0)
                    nc.sync.dma_start(out=d_out[:], in_=outsb[:])

    if postprocess:
        _split_waits(nc)
    mybir.codegen_inst_isa_subclasses(nc)
    return nc


_NC_CACHE = {}
MODE = "dev"                      # "dev": on-device gather; "host": host gather


def _get_nc(reps=1, use_cc=True, mode=None):
    mode = MODE if mode is None else mode
    key = (reps, use_cc, mode)
    if key not in _NC_CACHE:
        _NC_CACHE[key] = _build_nc(reps, use_cc, mode)
    return _NC_CACHE[key]


def _prep_inputs(text, start_times, emb, conv_w, conv_b, W, b, mode=None):
    mode = MODE if mode is None else mode
    bf16 = ml_dtypes.bfloat16
    text = np.asarray(text)[0]              # [N, L]
    st = np.asarray(start_times)[0].astype(np.int64)   # [N]
    emb = np.asarray(emb, dtype=np.float32)
    conv_w = np.asarray(conv_w, dtype=np.float32)
    conv_b = np.asarray(conv_b, dtype=np.float32)
    W = np.asarray(W, dtype=np.float32)
    b = np.asarray(b, dtype=np.float32)

    embf = np.ascontiguousarray(emb)        # [V, 64] f32 -> 256B rows

    w01 = np.zeros((128, H), dtype=bf16)
    w01[:64, :] = conv_w[:, :, 0].T.astype(bf16)
    w01[64:, :] = conv_w[:, :, 1].T.astype(bf16)
    w2 = np.ascontiguousarray(conv_w[:, :, 2].T.astype(bf16))
    convb2 = np.ascontiguousarray(conv_b.reshape(2, 128).T.astype(np.float32))

    iota = np.tile(np.arange(S, dtype=np.float32), (128, 1))
    wf = np.tile(W[1:H + 1, 0], (128, 1)).astype(np.float32)
    w0 = np.full((128, 1), W[0, 0], np.float32)
    bsc = np.full((1, 1), b[0], np.float32)

    delta_g = np.concatenate([[0.0], np.diff(st).astype(np.float32)]).astype(np.float32)

    tok = text.astype(np.int16)             # V=30000 < 2**15
    in_maps = []
    for c in range(NCORES):
        sl = slice(c * NC_NOTES, (c + 1) * NC_NOTES)
        im = {
            "stf": np.ascontiguousarray(
                st[sl].astype(np.float32).reshape(NCHUNK, 128).T),
            "delta": np.ascontiguousarray(
                delta_g[sl].reshape(NCHUNK, 128).T),
            "w01": w01,
            "w2": w2,
            "convb2": convb2,
            "iota": iota,
            "wf": wf,
            "w0": w0,
            "bsc": bsc,
        }
        if mode == "dev":
            t = tok[sl].reshape(-1)             # [NTOK]
            idx = np.zeros((128, NTOK // 16), np.int16)
            for bidx in range(NBLK):
                w = t[bidx * BLK_TOK:(bidx + 1) * BLK_TOK].reshape(BLK_TOK // 16, 16).T
                idx[:, bidx * (BLK_TOK // 16):(bidx + 1) * (BLK_TOK // 16)] = \
                    np.tile(w, (8, 1))
            im["embf"] = embf
            im["idx"] = np.ascontiguousarray(idx)
        else:
            t64 = text[sl].reshape(-1).astype(np.int64)   # [NTOK]
            xeT = emb[t64].T                              # [64, NTOK] f32 view
            xs = np.zeros((NBLK, 128, BLK_TOK), dtype=np.float32)
            for bidx in range(NBLK):
                blk = xeT[:, bidx * BLK_TOK:(bidx + 1) * BLK_TOK]
                xs[bidx, 0:64, :] = blk                    # k0
                if bidx < NBLK - 1:
                    xs[bidx, 64:128, :] = xeT[:, bidx * BLK_TOK + 1:
                                              (bidx + 1) * BLK_TOK + 1]
                else:
                    xs[bidx, 64:128, :-1] = xeT[:, bidx * BLK_TOK + 1:]
            im["xs"] = xs.astype(bf16)
        in_maps.append(im)
    return in_maps


def kernel(**inputs) -> np.ndarray:
    nc = _get_nc()
    in_maps = _prep_inputs(**inputs)
    res = run_bass_kernel_spmd(nc, in_maps, list(range(NCORES))).results
    return res[0]["out"].reshape(S, 1).astype(np.float32)


if __name__ == "__main__":
    import jax
    import reference
    cpu = jax.devices("cpu")[0]
    with jax.default_device(cpu):
        ins = {k: np.asarray(v) for k, v in reference.setup_inputs().items()}
        exp = np.asarray(reference.reference(**reference.setup_inputs()))
    got = kernel(**ins)
    err = np.abs(got - exp).max()
    rel = err / max(np.abs(exp).max(), 1e-9)
    print("max abs err:", err, "rel:", rel)


# revision 12
# speedup vs baseline: 1.4982x; 1.4982x over previous
"""Trainium2 Bass kernel v2 for nn_BaseLineModel (segment_reduce).

Changes vs v1 baseline:
- Embedding gather uses NON-transpose dma_gather of exact 256B fp32 rows
  (token -> one contiguous 256B descriptor) instead of transpose-mode
  (which wrote 2B per partition per token -> read-modify-write death,
  ~670ms/pass). Token-major gather output is transposed to feature-major
  on-device with paired PE transposes; the k0|k1 stacked bf16 conv
  operand is built during the PSUM->SBUF copies (all on ACT; DVE is the
  busier engine with the maxpool). Gathers ride 4 SWDGE queues
  (multi-packet, 8192 idx/call) - queue count scales random-read BW.
- The per-day linear layer is applied BEFORE the segment reduction:
  per-note dot = [delta|feats].W computed on DVE, then segment-sum of
  [dot, 1] via one-hot matmuls -> [2, 1024] partials, AllReduce (8KB)
  instead of ReduceScatter of [1024, 258] (1MB).
- Finalize: mean = dot_sum/max(cnt,1), sigmoid(mean + b) on a [1, 1024]
  row; host takes core 0's full output.
"""

import numpy as np
import ml_dtypes

import concourse.bass as bass
import concourse.mybir as mybir
import concourse.tile as tile
from concourse.bass_utils import run_bass_kernel_spmd
from concourse import library_config
from concourse.masks import make_identity

# ---- problem dims (hardcoded per task contract) ----
N, L, E, H, K, V, S = 16384, 64, 64, 256, 3, 30000, 1024
NCORES = 8
NC_NOTES = N // NCORES            # 2048 notes per core
NTOK = NC_NOTES * L               # 131072 tokens per core
BLK_TOK = 8192                    # tokens per processing block
NBLK = NTOK // BLK_TOK            # 16
GCHUNK = 4096                     # tokens per dma_gather call: two calls per
                                  # block keep all 4 SWDGE queues busy and let
                                  # transposes start after the first half
NGC = BLK_TOK // GCHUNK           # gather calls per block
# >64 descs/engine (GCHUNK>1008) cannot ride one packet; use multi-packet then
SINGLE_PACKET = (GCHUNK // 16 + 1) <= 64
NSUB = BLK_TOK // 128             # 64 sub-blocks of 128 tokens
NPAIR = NSUB // 2                 # 32 transpose pairs
NGRP = BLK_TOK // 512             # 16 conv groups per block
NCHUNK = NC_NOTES // 128          # 16 note-chunks
TMAX = L - K + 1                  # 62 valid conv positions

# scheduling knobs (A/B-tunable): gather pool, transpose PSUM, conv PSUM,
# xstack pool buffer counts
POOL_BUFS = (4, 4, 3, 2)

_SPLIT_MAXW = 1


def _split_waits(nc, maxw=_SPLIT_MAXW):
    """This walrus build rejects >1 sync wait per instruction; move extras
    onto preceding same-engine NOPs (sequencer order preserves semantics)."""
    for bb in nc.main_func.blocks:
        out = []
        for inst in bb.instructions:
            si = inst.sync_info
            waits = list(si.on_wait) if (si is not None and si.on_wait) else []
            if len(waits) > maxw:
                rest = waits[:-maxw]
                si.on_wait = waits[-maxw:]
                for i in range(0, len(rest), maxw):
                    out.append(mybir.InstNoOp(
                        name=f"{inst.name}-wsplit{i}",
                        sync_info=mybir.SyncInfo(on_wait=rest[i:i + maxw], on_update=[]),
                        bass_nofuse=True,
                        engine=inst.engine,
                    ))
            out.append(inst)
        bb.instructions = out


def _build_nc(reps=1, use_cc=True, mode="dev", postprocess=True):
    f32 = mybir.dt.float32
    bf16 = mybir.dt.bfloat16
    i16 = mybir.dt.int16
    Relu = mybir.ActivationFunctionType.Relu
    Copy = mybir.ActivationFunctionType.Copy
    Sigmoid = mybir.ActivationFunctionType.Sigmoid

    nc = bass.Bass(num_swdge_queues=4 if mode == "dev" else 1)
    if mode == "dev":
        d_embf = nc.declare_dram_parameter("embf", [V, E], f32, isOutput=False)
        d_idx = nc.declare_dram_parameter("idx", [128, NTOK // 16], i16, isOutput=False)
    else:
        d_x = nc.declare_dram_parameter("xs", [NBLK, 128, BLK_TOK], bf16, isOutput=False)
    d_stf = nc.declare_dram_parameter("stf", [128, NCHUNK], f32, isOutput=False)
    d_delta = nc.declare_dram_parameter("delta", [128, NCHUNK], f32, isOutput=False)
    d_w01 = nc.declare_dram_parameter("w01", [128, H], bf16, isOutput=False)
    d_w2 = nc.declare_dram_parameter("w2", [64, H], bf16, isOutput=False)
    d_cb = nc.declare_dram_parameter("convb2", [128, 2], f32, isOutput=False)
    d_iota = nc.declare_dram_parameter("iota", [128, S], f32, isOutput=False)
    d_wf = nc.declare_dram_parameter("wf", [128, H], f32, isOutput=False)
    d_w0 = nc.declare_dram_parameter("w0", [128, 1], f32, isOutput=False)
    d_bsc = nc.declare_dram_parameter("bsc", [1, 1], f32, isOutput=False)
    d_out = nc.declare_dram_parameter("out", [1, S], f32, isOutput=True)
    part = nc.dram_tensor("part", [2, S], f32)
    ar_out = nc.dram_tensor("ar_out", [2, S], f32)

    with tile.TileContext(nc) as tc:
        if mode == "dev":
            nc.gpsimd.load_library(library_config.mlp)
            nidx_reg_cm = nc.gpsimd.register("nidx")
            nidx_reg = nidx_reg_cm.__enter__()
            nc.gpsimd.reg_mov(nidx_reg, GCHUNK)
        with (
            tc.tile_pool(name="cst", bufs=1) as cp,
            tc.tile_pool(name="feat", bufs=1) as fp,
        ):
            w01_sb = cp.tile([128, H], bf16)
            w2_sb = cp.tile([64, H], bf16)
            cb_sb = cp.tile([128, 2], f32)
            iota_sb = cp.tile([128, S], f32)
            wf_sb = cp.tile([128, H], f32)
            w0_sb = cp.tile([128, 1], f32)
            bsc_sb = cp.tile([1, 1], f32)
            stf_sb = cp.tile([128, NCHUNK], f32)
            delta_sb = cp.tile([128, NCHUNK], f32)
            ones_sb = cp.tile([128, 1], f32)
            nc.vector.memset(ones_sb[:], 1.0)
            if mode == "dev":
                idx_sb = cp.tile([128, NTOK // 16], i16)
            ident = cp.tile([128, 128], f32)
            nc.sync.dma_start(out=w01_sb[:], in_=d_w01[:])
            nc.sync.dma_start(out=w2_sb[:], in_=d_w2[:])
            nc.sync.dma_start(out=cb_sb[:], in_=d_cb[:])
            nc.sync.dma_start(out=iota_sb[:], in_=d_iota[:])
            nc.sync.dma_start(out=wf_sb[:], in_=d_wf[:])
            nc.sync.dma_start(out=w0_sb[:], in_=d_w0[:])
            nc.sync.dma_start(out=bsc_sb[:], in_=d_bsc[:])
            nc.sync.dma_start(out=stf_sb[:], in_=d_stf[:])
            nc.sync.dma_start(out=delta_sb[:], in_=d_delta[:])
            if mode == "dev":
                nc.sync.dma_start(out=idx_sb[:], in_=d_idx[:])
            make_identity(nc, ident[:])

            for _rep in range(reps):
                feats = [fp.tile([128, NC_NOTES], f32, tag=f"feats{hh}",
                                 name=f"feats{hh}")
                         for hh in range(2)]

                # ---- P1: gather (token-major) + transpose-stack + conv + maxpool ----
                gp_b, tp_b, yp_b, xp_b = POOL_BUFS
                with (
                    tc.tile_pool(name="gath", bufs=gp_b) as gp,
                    tc.tile_pool(name="xstk", bufs=xp_b) as xp,
                    tc.tile_pool(name="tps", bufs=tp_b, space="PSUM") as tp,
                    tc.tile_pool(name="ypsum", bufs=yp_b, space="PSUM") as yp,
                ):
                    for b in range(NBLK):
                        if mode == "host":
                            xstack = xp.tile([128, BLK_TOK], bf16, tag="xs")
                            nc.sync.dma_start(out=xstack[:], in_=d_x[b])
                        else:
                            x_tok = gp.tile([128, NSUB * E], f32, tag="xtok")
                            for gc in range(NGC):
                                col0 = gc * (GCHUNK // 128) * E
                                icol0 = b * (BLK_TOK // 16) + gc * (GCHUNK // 16)
                                nc.gpsimd.dma_gather(
                                    out_ap=x_tok[:, col0:col0 + (GCHUNK // 128) * E]
                                        .rearrange("p (o n) -> p o n", n=E),
                                    in_ap=d_embf[:],
                                    idxs_ap=idx_sb[:, icol0:icol0 + GCHUNK // 16],
                                    num_idxs=GCHUNK,
                                    num_idxs_reg=nidx_reg,
                                    elem_size=E,
                                    transpose=False,
                                    single_packet=SINGLE_PACKET,
                                    queue_num=(b * NGC + gc) % 4,
                                )
                            xstack = xp.tile([128, BLK_TOK], bf16, tag="xs")
                            # last k1 column is never produced (needs next
                            # block); zero it so the discarded tail is finite
                            nc.vector.memset(xstack[64:128, BLK_TOK - 1:BLK_TOK], 0.0)
                            for k in range(NPAIR):
                                t_ps = tp.tile([128, 128], f32, tag="t")
                                nc.tensor.transpose(out=t_ps[:],
                                                    in_=x_tok[:, k * 128:(k + 1) * 128],
                                                    identity=ident[:])
                                c_e = 2 * k * 128       # even sub-block col base
                                c_o = c_e + 128         # odd sub-block col base
                                # k0 rows (ACT engine)
                                nc.scalar.activation(out=xstack[0:64, c_e:c_e + 128],
                                                     in_=t_ps[0:64, :], func=Copy)
                                nc.scalar.activation(out=xstack[0:64, c_o:c_o + 128],
                                                     in_=t_ps[64:128, :], func=Copy)
                                # k1 rows, shifted left by one token (ACT too:
                                # DVE is the busier engine with the maxpool)
                                if k == 0:
                                    nc.scalar.activation(out=xstack[64:128, 0:127],
                                                         in_=t_ps[0:64, 1:128],
                                                         func=Copy)
                                else:
                                    nc.scalar.activation(
                                        out=xstack[64:128, c_e - 1:c_e + 127],
                                        in_=t_ps[0:64, :], func=Copy)
                                nc.scalar.activation(
                                    out=xstack[64:128, c_o - 1:c_o + 127],
                                    in_=t_ps[64:128, :], func=Copy)
                        for g in range(NGRP):
                            c0 = g * 512
                            for hh in range(2):
                                y_ps = yp.tile([128, 512], f32, tag="y")
                                nc.tensor.matmul(out=y_ps[:],
                                                 lhsT=w01_sb[:, hh * 128:(hh + 1) * 128],
                                                 rhs=xstack[:, c0:c0 + 512],
                                                 start=True, stop=False)
                                nc.tensor.matmul(out=y_ps[:, 0:510],
                                                 lhsT=w2_sb[:, hh * 128:(hh + 1) * 128],
                                                 rhs=xstack[0:64, c0 + 2:c0 + 512],
                                                 start=False, stop=True)
                                nc.vector.reduce_max(
                                    out=feats[hh][:, b * 128 + g * 8:b * 128 + g * 8 + 8],
                                    in_=y_ps[:].rearrange("p (n l) -> p n l", l=L)[:, :, 0:TMAX],
                                    axis=mybir.AxisListType.X)

                # ---- P2: relu(feats + conv_b) ----
                for hh in range(2):
                    nc.scalar.activation(out=feats[hh][:], in_=feats[hh][:],
                                         func=Relu, bias=cb_sb[:, hh:hh + 1], scale=1.0)

                # ---- P3+P4: note-major dot, one-hot segment matmuls ----
                with (
                    tc.tile_pool(name="seg", bufs=1) as ssp,
                    tc.tile_pool(name="tps2", bufs=2, space="PSUM") as tp2,
                    tc.tile_pool(name="segps", bufs=1, space="PSUM") as pp,
                ):
                    seg_ps = [[pp.tile([1, 512], f32, tag=f"seg{v}{h}",
                                       name=f"seg{v}{h}")
                               for h in range(2)] for v in range(2)]
                    for i in range(NCHUNK):
                        mainf = ssp.tile([128, H], f32, tag="mainf")
                        for hh in range(2):
                            t2 = tp2.tile([128, 128], f32, tag="t2")
                            nc.tensor.transpose(out=t2[:],
                                                in_=feats[hh][:, i * 128:(i + 1) * 128],
                                                identity=ident[:])
                            nc.vector.tensor_copy(
                                out=mainf[:, hh * 128:(hh + 1) * 128], in_=t2[:])
                        prod = ssp.tile([128, H], f32, tag="prod")
                        nc.vector.tensor_tensor(out=prod[:], in0=mainf[:], in1=wf_sb[:],
                                                op=mybir.AluOpType.mult)
                        dotc = ssp.tile([128, 1], f32, tag="dotc")
                        nc.vector.reduce_sum(out=dotc[:], in_=prod[:],
                                             axis=mybir.AxisListType.X)
                        dterm = ssp.tile([128, 1], f32, tag="dterm")
                        nc.vector.tensor_tensor(out=dterm[:], in0=delta_sb[:, i:i + 1],
                                                in1=w0_sb[:], op=mybir.AluOpType.mult)
                        nc.vector.tensor_add(out=dotc[:], in0=dotc[:], in1=dterm[:])
                        oh = ssp.tile([128, S], f32, tag="oh")
                        nc.vector.tensor_tensor(out=oh[:],
                                                in0=stf_sb[:, i:i + 1].to_broadcast([128, S]),
                                                in1=iota_sb[:],
                                                op=mybir.AluOpType.is_equal)
                        for h in range(2):
                            nc.tensor.matmul(out=seg_ps[0][h][:],
                                             lhsT=dotc[:],
                                             rhs=oh[:, h * 512:(h + 1) * 512],
                                             start=(i == 0), stop=(i == NCHUNK - 1))
                            nc.tensor.matmul(out=seg_ps[1][h][:],
                                             lhsT=ones_sb[:],
                                             rhs=oh[:, h * 512:(h + 1) * 512],
                                             start=(i == 0), stop=(i == NCHUNK - 1))
                    sd = fp.tile([1, S], f32, tag="segd")
                    sc = fp.tile([1, S], f32, tag="segc")
                    for h in range(2):
                        nc.vector.tensor_copy(out=sd[:, h * 512:(h + 1) * 512],
                                              in_=seg_ps[0][h][:])
                        nc.vector.tensor_copy(out=sc[:, h * 512:(h + 1) * 512],
                                              in_=seg_ps[1][h][:])
                    nc.sync.dma_start(out=part[0:1, :], in_=sd[:])
                    nc.sync.dma_start(out=part[1:2, :], in_=sc[:])

                # ---- P5: cross-core reduce + finalize ----
                if use_cc:
                    with tc.tile_critical():
                        with nc.semaphore(f"cc_sem{_rep}") as cc_sem:
                            nc.gpsimd.collective_compute(
                                "AllReduce", mybir.AluOpType.add,
                                replica_groups=[list(range(NCORES))],
                                ins=[part[:]], outs=[ar_out[:]],
                            ).then_inc(cc_sem, 1)
                            nc.gpsimd.wait_ge(cc_sem, 1)
                with tc.tile_pool(name="fin", bufs=1) as fin:
                    if use_cc:
                        fd = fin.tile([1, S], f32)
                        fc = fin.tile([1, S], f32)
                        nc.sync.dma_start(out=fd[:], in_=ar_out[0:1, :])
                        nc.sync.dma_start(out=fc[:], in_=ar_out[1:2, :])
                    else:
                        fd, fc = sd, sc
                    cnt = fin.tile([1, S], f32)
                    nc.vector.tensor_scalar_max(out=cnt[:], in0=fc[:], scalar1=1.0)
                    rcp = fin.tile([1, S], f32)
                    nc.vector.reciprocal(out=rcp[:], in_=cnt[:])
                    v = fin.tile([1, S], f32)
                    nc.vector.tensor_tensor(out=v[:], in0=fd[:], in1=rcp[:],
                                            op=mybir.AluOpType.mult)
                    outsb = fin.tile([1, S], f32)
                    nc.scalar.activation(out=outsb[:], in_=v[:], func=Sigmoid,
                                         bias=bsc_sb[:], scale=1.0)
                    nc.sync.dma_start(out=d_out[:], in_=outsb[:])

    if postprocess:
        _split_waits(nc)
    mybir.codegen_inst_isa_subclasses(nc)
    return nc


_NC_CACHE = {}
MODE = "dev"                      # "dev": on-device gather; "host": host gather


def _get_nc(reps=1, use_cc=True, mode=None):
    mode = MODE if mode is None else mode
    key = (reps, use_cc, mode)
    if key not in _NC_CACHE:
        _NC_CACHE[key] = _build_nc(reps, use_cc, mode)
    return _NC_CACHE[key]


def _prep_inputs(text, start_times, emb, conv_w, conv_b, W, b, mode=None):
    mode = MODE if mode is None else mode
    bf16 = ml_dtypes.bfloat16
    text = np.asarray(text)[0]              # [N, L]
    st = np.asarray(start_times)[0].astype(np.int64)   # [N]
    emb = np.asarray(emb, dtype=np.float32)
    conv_w = np.asarray(conv_w, dtype=np.float32)
    conv_b = np.asarray(conv_b, dtype=np.float32)
    W = np.asarray(W, dtype=np.float32)
    b = np.asarray(b, dtype=np.float32)

    embf = np.ascontiguousarray(emb)        # [V, 64] f32 -> 256B rows

    w01 = np.zeros((128, H), dtype=bf16)
    w01[:64, :] = conv_w[:, :, 0].T.astype(bf16)
    w01[64:, :] = conv_w[:, :, 1].T.astype(bf16)
    w2 = np.ascontiguousarray(conv_w[:, :, 2].T.astype(bf16))
    convb2 = np.ascontiguousarray(conv_b.reshape(2, 128).T.astype(np.float32))

    iota = np.tile(np.arange(S, dtype=np.float32), (128, 1))
    wf = np.tile(W[1:H + 1, 0], (128, 1)).astype(np.float32)
    w0 = np.full((128, 1), W[0, 0], np.float32)
    bsc = np.full((1, 1), b[0], np.float32)

    delta_g = np.concatenate([[0.0], np.diff(st).astype(np.float32)]).astype(np.float32)

    tok = text.astype(np.int16)             # V=30000 < 2**15
    in_maps = []
    for c in range(NCORES):
        sl = slice(c * NC_NOTES, (c + 1) * NC_NOTES)
        im = {
            "stf": np.ascontiguousarray(
                st[sl].astype(np.float32).reshape(NCHUNK, 128).T),
            "delta": np.ascontiguousarray(
                delta_g[sl].reshape(NCHUNK, 128).T),
            "w01": w01,
            "w2": w2,
            "convb2": convb2,
            "iota": iota,
            "wf": wf,
            "w0": w0,
            "bsc": bsc,
        }
        if mode == "dev":
            t = tok[sl].reshape(-1)             # [NTOK]
            idx = np.zeros((128, NTOK // 16), np.int16)
            for bidx in range(NBLK):
                w = t[bidx * BLK_TOK:(bidx + 1) * BLK_TOK].reshape(BLK_TOK // 16, 16).T
                idx[:, bidx * (BLK_TOK // 16):(bidx + 1) * (BLK_TOK // 16)] = \
                    np.tile(w, (8, 1))
            im["embf"] = embf
            im["idx"] = np.ascontiguousarray(idx)
        else:
            t64 = text[sl].reshape(-1).astype(np.int64)   # [NTOK]
            xeT = emb[t64].T                              # [64, NTOK] f32 view
            xs = np.zeros((NBLK, 128, BLK_TOK), dtype=np.float32)
            for bidx in range(NBLK):
                blk = xeT[:, bidx * BLK_TOK:(bidx + 1) * BLK_TOK]
                xs[bidx, 0:64, :] = blk                    # k0
                if bidx < NBLK - 1:
                    xs[bidx, 64:128, :] = xeT[:, bidx * BLK_TOK + 1:
                                              (bidx + 1) * BLK_TOK + 1]
                else:
                    xs[bidx, 64:128, :-1] = xeT[:, bidx * BLK_TOK + 1:]
            im["xs"] = xs.astype(bf16)
        in_maps.append(im)
    return in_maps


def kernel(**inputs) -> np.ndarray:
    nc = _get_nc()
    in_maps = _prep_inputs(**inputs)
    res = run_bass_kernel_spmd(nc, in_maps, list(range(NCORES))).results
    return res[0]["out"].reshape(S, 1).astype(np.float32)


if __name__ == "__main__":
    import jax
    import reference
    cpu = jax.devices("cpu")[0]
    with jax.default_device(cpu):
        ins = {k: np.asarray(v) for k, v in reference.setup_inputs().items()}
        exp = np.asarray(reference.reference(**reference.setup_inputs()))
    got = kernel(**ins)
    err = np.abs(got - exp).max()
    rel = err / max(np.abs(exp).max(), 1e-9)
    print("max abs err:", err, "rel:", rel)
